# revision 36
# baseline (speedup 1.0000x reference)
"""ExpanderGIN message-passing kernel for 8 Trainium2 NeuronCores.

out = relu((x + segment_sum(x[src], dst)) @ W.T + b)

Strategy (graph-parallel, no collectives), fp16 message path:
  - Destination nodes are sharded 8 ways (12500 nodes/core, 98 tiles of
    128 slots). A 4-D bin-packer assigns nodes to tiles so per-(tile,
    src-quarter) in-degree sums land just under 128-block boundaries
    (~3% padding vs ~28% for naive assignment); the block budget B_star
    is shared across cores (SPMD program), per-core loads fit under it.
  - Edge rows are fetched with the SWDGE dma_gather custom instruction
    from a replicated fp16 copy of x. The int16 index limit forces 4
    quarter-tables of 25000 rows. Slots are laid out quarter-major
    (q -> tile -> blocks) and each quarter's block stream is chopped
    into full 1024-index gather instructions independent of tile
    boundaries (80 instructions/rep), one SWDGE queue per quarter,
    issued in waves 1-2 ahead of consumption. single_packet=False.
  - Aggregation: for each 128-edge chunk, a one-hot(dst) matrix [128
    edges, 128 slots] in fp16 is built by comparing an fp16 iota row
    against per-edge dst columns (one broadcast tensor_tensor per gather
    instruction), then TensorE computes agg^T += gx^T @ onehot in PSUM
    (f32). fp16 operands run the PE at 1 cycle/row (vs 4 for f32) with
    fast weight loads.
  - The self term x is added from a host-side permuted/transposed fp16
    copy of x, fused into the PSUM->SBUF eviction add (output fp16 =
    MLP matmul input). xt and out pack two 128-slot tiles per DRAM row
    so every streaming DMA descriptor moves 512B contiguous (256B rows
    fall off HBM line rate).
  - MLP: psum_out[nodes, outfeat] = ht.T @ W^T in fp16 (+ a K=1 bias
    matmul only when b != 0), then ReLU on the scalar engine -> fp16
    out, cast to f32 on the host.
  - Bench note: wall-clock dispatch through axon is ~70ms with ms-level
    jitter, so test.py measures the repeat-slope inside a device-side
    For_i loop (loop=200, repeat 1 vs 3): dispatch cost and loop barrier
    overhead cancel, leaving per-rep kernel time.
"""

import numpy as np

N = 100000
E = 625000
D = 128
NC = 8            # cores
NPC = N // NC     # 12500 nodes per core
P = 128
TPC = (NPC + P - 1) // P   # 98 tiles per core
SLOTS = TPC * P            # 12544 slots per core
NQ = 4                     # quarter tables (int16 index limit)
QROWS = N // NQ            # 25000

SCRATCH = 49152            # SWDGE ring: 3072 descs/queue = two full
                           # 1536-idx gathers in flight (gen overlaps drain)
MAXI = 1536                # >1024 idx needs single_packet=False (64-desc
                           # packet ceiling per engine); ring fits 2 instrs
MAXB = MAXI // P           # blocks per gather instruction

_f32 = np.float32
_f16 = np.float16


def _pack_tiles(dq):
    """4-D bin-packing: assign each core's nodes to 98 tiles of <=128 nodes
    so per-(tile, quarter) in-degree sums fit a shared block budget B_star
    [TPC, NQ] with minimal total padding. Returns (B_star, node_of)."""
    Tcq = dq.reshape(NC, NPC, NQ).sum(axis=1)          # [NC, NQ] totals
    need = (Tcq.max(axis=0) + P - 1) // P              # blocks per quarter

    for margin in (2, 3, 4, 6, 9):
        Kq = need + margin
        # distribute Kq[q] blocks over TPC tiles (>=1 each), extras spread
        # round-robin so per-tile totals stay balanced
        B_star = np.ones((TPC, NQ), np.int64)
        extras = []
        for q in range(NQ):
            extras += [q] * int(Kq[q] - TPC)
        for i, q in enumerate(extras):
            B_star[i % TPC, q] += 1
        caps0 = B_star * P

        node_of = np.full((NC, SLOTS), -1, np.int64)
        ok = True
        for c in range(NC):
            nodes = np.arange(c * NPC, (c + 1) * NPC)
            d = dq[nodes]                              # [NPC, NQ]
            order = np.argsort(-d.sum(1), kind="stable")
            rem = caps0.astype(np.int64).copy()
            cnt = np.zeros(TPC, np.int64)
            assign = np.empty(NPC, np.int64)
            for n in order:
                dn = d[n]
                feas = (cnt < P) & np.all(rem >= dn, axis=1)
                if not feas.any():
                    ok = False
                    break
                score = (rem - dn).min(axis=1) * 1024 + (P - cnt)
                score[~feas] = -1
                t = int(np.argmax(score))
                assign[n] = t
                rem[t] -= dn
                cnt[t] += 1
            if not ok:
                break
            fill = np.zeros(TPC, np.int64)
            for n in range(NPC):
                t = assign[n]
                node_of[c, t * P + fill[t]] = nodes[n]
                fill[t] += 1
        if ok:
            return B_star, node_of
    raise RuntimeError("tile packing failed at all margins")


def _preprocess(edge_index):
    """Shard edges. Returns per-core host arrays + layout metadata."""
    src = np.asarray(edge_index[0]).astype(np.int64)
    dst = np.asarray(edge_index[1]).astype(np.int64)

    eq = src // QROWS
    # per-node quarter in-degree
    dq = np.bincount(dst * NQ + eq, minlength=N * NQ).reshape(N, NQ)

    Btq, node_of = _pack_tiles(dq)
    slot_of = np.empty(N, np.int64)
    for c in range(NC):
        m = node_of[c] >= 0
        slot_of[node_of[c][m]] = np.nonzero(m)[0]

    ec = dst // NPC
    eslot = slot_of[dst]
    et = eslot // P
    epos = (eslot % P).astype(_f16)
    eqidx = (src % QROWS).astype(np.int16)

    key = (ec * TPC + et) * NQ + eq

    # quarter-major slot layout: q -> t -> blocks. Each quarter's block
    # stream is chopped into full MAXB-block gather instructions
    # independent of tile boundaries.
    slot_start = np.zeros((TPC, NQ), np.int64)
    qbase = np.zeros(NQ + 1, np.int64)
    pos = 0
    for q in range(NQ):
        qbase[q] = pos
        for t in range(TPC):
            slot_start[t, q] = pos
            pos += Btq[t, q] * P
    qbase[NQ] = pos
    S_total = pos
    assert S_total % 128 == 0

    # rank of each edge within its (c,t,q) group
    perm = np.argsort(key, kind="stable")
    gstart = np.concatenate([[0], np.cumsum(np.bincount(key, minlength=NC * TPC * NQ))])[:-1]
    ranks = np.empty(len(perm), np.int64)
    ranks[perm] = np.arange(len(perm)) - gstart[key[perm]]

    flat = slot_start[et, eq] + ranks   # slot within core's flat layout

    qidx_slots = np.zeros((NC, S_total), np.int16)
    dst_slots = np.full((NC, S_total), 999.0, _f16)
    qidx_slots[ec, flat] = eqidx
    dst_slots[ec, flat] = epos

    idx16 = np.empty((NC, P, S_total // 16), np.int16)
    dstl = np.empty((NC, P, S_total // 128), _f16)
    for c in range(NC):
        wrapped = qidx_slots[c].reshape(-1, 16).T   # [16, S/16]
        idx16[c] = np.tile(wrapped, (8, 1))
        dstl[c] = dst_slots[c].reshape(-1, 128).T   # [128, S/128]

    return {
        "Btq": Btq,
        "slot_start": slot_start,
        "S_total": S_total,
        "qbase": qbase,
        "idx16": idx16,
        "dstl": dstl,
        "node_of": node_of,
    }


def _build_program(Btq, slot_start, S_total, qbase, has_bias, repeat=1, loop=1,
                   ablate=""):
    import concourse.bacc as bacc
    import concourse.mybir as mybir
    import concourse.tile as tile
    from contextlib import ExitStack, nullcontext

    f32 = mybir.dt.float32
    f16 = mybir.dt.float16
    nc = bacc.Bacc(
        "TRN2", target_bir_lowering=False, debug=False, num_devices=NC,
        num_swdge_queues=4, dynamic_dma_scratch_size=SCRATCH,
    )

    # xt/out pack two 128-slot tiles per DRAM row pair-wise so every DMA
    # descriptor moves 512B contiguous (fp16 256B rows fall off HBM line
    # rate: writes below 512B do read-modify-write).
    x_d = nc.dram_tensor("x16", [N, D], f16, kind="ExternalInput")
    xt_d = nc.dram_tensor("xt", [SLOTS // 2, 2 * D], f16, kind="ExternalInput")
    idx_d = nc.dram_tensor("idx16", [P, S_total // 16], mybir.dt.int16, kind="ExternalInput")
    dst_d = nc.dram_tensor("dstl", [P, S_total // 128], f16, kind="ExternalInput")
    wt_d = nc.dram_tensor("wt", [D, D], f16, kind="ExternalInput")
    b_d = nc.dram_tensor("bias", [1, D], f32, kind="ExternalInput")
    out_d = nc.dram_tensor("out", [SLOTS // 2, 2 * D], f16, kind="ExternalOutput")

    with tile.TileContext(nc) as tc, ExitStack() as ctx:
        const = ctx.enter_context(tc.tile_pool(name="const", bufs=1))
        gxp = ctx.enter_context(tc.tile_pool(name="gx", bufs=14))
        ohp = ctx.enter_context(tc.tile_pool(name="oh", bufs=12))
        xtp = ctx.enter_context(tc.tile_pool(name="xt", bufs=3))
        htp = ctx.enter_context(tc.tile_pool(name="ht", bufs=3))
        obp = ctx.enter_context(tc.tile_pool(name="ob", bufs=3))
        pag = ctx.enter_context(tc.tile_pool(name="pagg", bufs=4, space="PSUM"))
        pou = ctx.enter_context(tc.tile_pool(name="pout", bufs=2, space="PSUM"))

        idx_t = const.tile([P, S_total // 16], mybir.dt.int16)
        nc.sync.dma_start(out=idx_t[:], in_=idx_d[:])
        dst_t = const.tile([P, S_total // 128], f16)
        nc.sync.dma_start(out=dst_t[:], in_=dst_d[:])
        wt_t = const.tile([D, D], f16)
        nc.sync.dma_start(out=wt_t[:], in_=wt_d[:])
        if has_bias:
            b_t = const.tile([1, D], f32)
            nc.sync.dma_start(out=b_t[:], in_=b_d[:])
            ones_t = const.tile([1, D], f32)
            nc.vector.memset(ones_t[:], 1.0)
        iota_i = const.tile([P, P], mybir.dt.int32)
        nc.gpsimd.iota(iota_i[:], pattern=[[1, P]], base=0, channel_multiplier=0)
        iota_f = const.tile([P, P], f16)
        nc.vector.tensor_copy(out=iota_f[:], in_=iota_i[:])

        maxB = int(Btq.sum(axis=0).max())
        cgx = const.tile([P, maxB, P], f16) if "no_gather" in ablate else None
        coh = const.tile([P, maxB, P], f16) if "no_onehot" in ablate else None
        if cgx is not None:
            nc.vector.memset(cgx[:], 0.25)
        if coh is not None:
            nc.vector.memset(coh[:], 0.0)

        # per-quarter gather instruction schedule: chop each quarter's block
        # stream [qbase[q]/P, qbase[q+1]/P) into MAXB-block instructions.
        qinstr = []  # [q][i] = (c0, nblk)  (chunk-col base, block count)
        for q in range(NQ):
            b0 = int(qbase[q]) // P
            b1 = int(qbase[q + 1]) // P
            qinstr.append(
                [(c, min(MAXB, b1 - c)) for c in range(b0, b1, MAXB)]
            )
        nwave = max(len(qi) for qi in qinstr)
        # chunk col -> (wave, offset within that instruction's gx tile)
        chunk_loc = {}
        for q in range(NQ):
            for w, (c0, nblk) in enumerate(qinstr[q]):
                for j in range(nblk):
                    chunk_loc[c0 + j] = (q, w, j)

        qn = 0
        with (tc.For_i(0, loop) if loop > 1 else nullcontext()):
          for _rep in range(repeat):
            gx_tiles = {}
            oh_tiles = {}

            def issue_wave(w):
                for q in range(NQ):
                    if w >= len(qinstr[q]):
                        continue
                    c0, nblk = qinstr[q][w]
                    if cgx is None:
                        gx = gxp.tile([P, nblk, P], f16, tag="gx")
                        nidx = nblk * P
                        nc.gpsimd.dma_gather(
                            gx[:],
                            x_d[q * QROWS : (q + 1) * QROWS, :],
                            idx_t[:, c0 * 8 : c0 * 8 + nidx // 16],
                            nidx,
                            nidx,
                            D,
                            queue_num=q,
                            single_packet=False,
                        )
                    else:
                        gx = cgx[:, :nblk, :]
                    if "gather_only" not in ablate:
                        if coh is None:
                            oh = ohp.tile([P, nblk, P], f16, tag="oh")
                            nc.vector.tensor_tensor(
                                out=oh[:],
                                in0=iota_f[:].unsqueeze(1).to_broadcast([P, nblk, P]),
                                in1=dst_t[:, c0 : c0 + nblk].unsqueeze(2).to_broadcast([P, nblk, P]),
                                op=mybir.AluOpType.is_equal,
                            )
                        else:
                            oh = coh[:, :nblk, :]
                        oh_tiles[(q, w)] = oh
                    gx_tiles[(q, w)] = gx

            # run 1 wave ahead of consumption (oh pool: 12 bufs = 3 waves)
            issue_wave(0)
            issue_wave(1)
            next_wave = 2
            if "gather_only" in ablate:
                for w in range(2, nwave):
                    issue_wave(w)
                continue
            for t in range(TPC):
                chunks = [(q, b) for q in range(NQ) for b in range(int(Btq[t, q]))]
                need_wave = max(
                    (chunk_loc[int(slot_start[t, q]) // P + b][1] for q, b in chunks),
                    default=-1,
                )
                while next_wave <= min(need_wave + 1, nwave - 1):
                    issue_wave(next_wave)
                    next_wave += 1
                if True:
                    half = t % 2
                    tp = t // 2
                    if half == 0:
                        xt_t = xtp.tile([P, 2 * P], f16, tag="xt")
                        nc.sync.dma_start(
                            out=xt_t[:], in_=xt_d[tp * P : (tp + 1) * P, :]
                        )
                        ob = obp.tile([P, 2 * P], f16, tag="ob")
                        cur_xt, cur_ob = xt_t, ob
                    else:
                        xt_t, ob = cur_xt, cur_ob
                    psum = pag.tile([P, P], f32, space="PSUM", tag="pagg")
                    for i, (q, b) in enumerate(chunks):
                        _, w, boff = chunk_loc[int(slot_start[t, q]) // P + b]
                        nc.tensor.matmul(
                            out=psum[:],
                            lhsT=gx_tiles[(q, w)][:, boff, :],
                            rhs=oh_tiles[(q, w)][:, boff, :],
                            start=(i == 0),
                            stop=(i == len(chunks) - 1),
                        )
                    ht = htp.tile([P, P], f16, tag="ht")
                    if chunks:
                        # h^T = agg^T + x^T (self term)
                        nc.vector.tensor_tensor(
                            out=ht[:],
                            in0=psum[:],
                            in1=xt_t[:, half * P : (half + 1) * P],
                            op=mybir.AluOpType.add,
                        )
                    else:
                        nc.vector.tensor_copy(
                            out=ht[:], in_=xt_t[:, half * P : (half + 1) * P]
                        )
                    po = pou.tile([P, P], f32, space="PSUM", tag="pout")
                    if has_bias:
                        nc.tensor.matmul(out=po[:], lhsT=ht[:], rhs=wt_t[:], start=True, stop=False)
                        nc.tensor.matmul(out=po[:], lhsT=ones_t[:], rhs=b_t[:], start=False, stop=True)
                    else:
                        nc.tensor.matmul(out=po[:], lhsT=ht[:], rhs=wt_t[:], start=True, stop=True)
                    nc.scalar.activation(
                        ob[:, half * P : (half + 1) * P],
                        po[:],
                        mybir.ActivationFunctionType.Relu,
                    )
                    if half == 1:
                        nc.sync.dma_start(
                            out=out_d[tp * P : (tp + 1) * P, :], in_=ob[:]
                        )
    nc.compile()
    return nc


def _prepare(x, edge_index, W, b, repeat=1, loop=1):
    x = np.ascontiguousarray(np.asarray(x, dtype=_f32))
    W = np.asarray(W, dtype=_f32)
    b = np.asarray(b, dtype=_f32)
    pre = _preprocess(edge_index)
    has_bias = bool(np.any(b != 0))
    nc = _build_program(
        pre["Btq"], pre["slot_start"], pre["S_total"], pre["qbase"],
        has_bias, repeat=repeat, loop=loop,
    )
    x16 = x.astype(_f16)
    wt = np.ascontiguousarray(W.T.astype(_f16))
    brow = np.ascontiguousarray(b.reshape(1, D))
    node_of = pre["node_of"]
    in_maps = []
    for c in range(NC):
        nidx = np.where(node_of[c] < 0, 0, node_of[c])
        # x^T per tile pair: [TPC/2, D feat, 2*P nodes] -> [SLOTS/2, 2D]
        xt = np.ascontiguousarray(
            x16[nidx]
            .reshape(TPC // 2, 2, P, D)
            .transpose(0, 3, 1, 2)
            .reshape(SLOTS // 2, 2 * D)
        )
        in_maps.append(
            {
                "x16": x16,
                "xt": xt,
                "idx16": np.ascontiguousarray(pre["idx16"][c]),
                "dstl": np.ascontiguousarray(pre["dstl"][c]),
                "wt": wt,
                "bias": brow,
            }
        )
    return nc, in_maps, node_of


def _assemble(results, node_of):
    out = np.empty((N, D), _f32)
    for c in range(NC):
        oc = (
            results[c]["out"]
            .reshape(TPC // 2, P, 2, D)
            .transpose(0, 2, 1, 3)
            .reshape(SLOTS, D)
        )
        m = node_of[c] >= 0
        out[node_of[c][m]] = oc[m].astype(_f32)
    return out


def kernel(x, edge_index, W, b):
    from concourse.bass_utils import run_bass_kernel_spmd

    nc, in_maps, node_of = _prepare(x, edge_index, W, b)
    res = run_bass_kernel_spmd(nc, in_maps, core_ids=list(range(NC)))
    return _assemble(res.results, node_of)


# revision 37
# speedup vs baseline: 1.1491x; 1.1491x over previous
"""ExpanderGIN message-passing kernel for 8 Trainium2 NeuronCores.

out = relu((x + segment_sum(x[src], dst)) @ W.T + b)

Strategy (graph-parallel, no collectives), fp16 message path:
  - Destination nodes are sharded 8 ways (12500 nodes/core, 98 tiles of
    128 slots). A 4-D bin-packer assigns nodes to tiles so per-(tile,
    src-quarter) in-degree sums land just under 128-block boundaries
    (~3% padding vs ~28% for naive assignment); the block budget B_star
    is shared across cores (SPMD program), per-core loads fit under it.
  - Edge rows are fetched with the SWDGE dma_gather custom instruction
    from a replicated fp16 copy of x. The int16 index limit forces 4
    quarter-tables of 25000 rows. Slots are laid out quarter-major
    (q -> tile -> blocks) and each quarter's block stream is chopped
    into full 1024-index gather instructions independent of tile
    boundaries (80 instructions/rep), one SWDGE queue per quarter,
    issued in waves 1-2 ahead of consumption. single_packet=False.
  - Aggregation: for each 128-edge chunk, a one-hot(dst) matrix [128
    edges, 128 slots] in fp16 is built by comparing an fp16 iota row
    against per-edge dst columns (one broadcast tensor_tensor per gather
    instruction), then TensorE computes agg^T += gx^T @ onehot in PSUM
    (f32). fp16 operands run the PE at 1 cycle/row (vs 4 for f32) with
    fast weight loads.
  - The self term x is added from a host-side permuted/transposed fp16
    copy of x, fused into the PSUM->SBUF eviction add (output fp16 =
    MLP matmul input). xt and out pack two 128-slot tiles per DRAM row
    so every streaming DMA descriptor moves 512B contiguous (256B rows
    fall off HBM line rate).
  - MLP: psum_out[nodes, outfeat] = ht.T @ W^T in fp16 (+ a K=1 bias
    matmul only when b != 0), then ReLU on the scalar engine -> fp16
    out, cast to f32 on the host.
  - Bench note: wall-clock dispatch through axon is ~70ms with ms-level
    jitter, so test.py measures the repeat-slope inside a device-side
    For_i loop (loop=200, repeat 1 vs 3): dispatch cost and loop barrier
    overhead cancel, leaving per-rep kernel time.
"""

import numpy as np

N = 100000
E = 625000
D = 128
NC = 8            # cores
NPC = N // NC     # 12500 nodes per core
P = 128
TPC = (NPC + P - 1) // P   # 98 tiles per core
SLOTS = TPC * P            # 12544 slots per core
NQ = 4                     # quarter tables (int16 index limit)
QROWS = N // NQ            # 25000

SCRATCH = 32768            # SWDGE ring carveout (bytes/partition)
MAXI = 2048                # >1024 idx needs single_packet=False (64-desc
                           # packet ceiling per engine); 2048 = best measured
MAXB = MAXI // P           # blocks per gather instruction

_f32 = np.float32
_f16 = np.float16


def _pack_tiles(dq):
    """4-D bin-packing: assign each core's nodes to 98 tiles of <=128 nodes
    so per-(tile, quarter) in-degree sums fit a shared block budget B_star
    [TPC, NQ] with minimal total padding. Returns (B_star, node_of)."""
    Tcq = dq.reshape(NC, NPC, NQ).sum(axis=1)          # [NC, NQ] totals
    need = (Tcq.max(axis=0) + P - 1) // P              # blocks per quarter

    for margin in (2, 3, 4, 6, 9):
        Kq = need + margin
        # distribute Kq[q] blocks over TPC tiles (>=1 each), extras spread
        # round-robin so per-tile totals stay balanced
        B_star = np.ones((TPC, NQ), np.int64)
        extras = []
        for q in range(NQ):
            extras += [q] * int(Kq[q] - TPC)
        for i, q in enumerate(extras):
            B_star[i % TPC, q] += 1
        caps0 = B_star * P

        node_of = np.full((NC, SLOTS), -1, np.int64)
        ok = True
        for c in range(NC):
            nodes = np.arange(c * NPC, (c + 1) * NPC)
            d = dq[nodes]                              # [NPC, NQ]
            order = np.argsort(-d.sum(1), kind="stable")
            rem = caps0.astype(np.int64).copy()
            cnt = np.zeros(TPC, np.int64)
            assign = np.empty(NPC, np.int64)
            for n in order:
                dn = d[n]
                feas = (cnt < P) & np.all(rem >= dn, axis=1)
                if not feas.any():
                    ok = False
                    break
                score = (rem - dn).min(axis=1) * 1024 + (P - cnt)
                score[~feas] = -1
                t = int(np.argmax(score))
                assign[n] = t
                rem[t] -= dn
                cnt[t] += 1
            if not ok:
                break
            fill = np.zeros(TPC, np.int64)
            for n in range(NPC):
                t = assign[n]
                node_of[c, t * P + fill[t]] = nodes[n]
                fill[t] += 1
        if ok:
            return B_star, node_of
    raise RuntimeError("tile packing failed at all margins")


def _preprocess(edge_index):
    """Shard edges. Returns per-core host arrays + layout metadata."""
    src = np.asarray(edge_index[0]).astype(np.int64)
    dst = np.asarray(edge_index[1]).astype(np.int64)

    eq = src // QROWS
    # per-node quarter in-degree
    dq = np.bincount(dst * NQ + eq, minlength=N * NQ).reshape(N, NQ)

    Btq, node_of = _pack_tiles(dq)
    slot_of = np.empty(N, np.int64)
    for c in range(NC):
        m = node_of[c] >= 0
        slot_of[node_of[c][m]] = np.nonzero(m)[0]

    ec = dst // NPC
    eslot = slot_of[dst]
    et = eslot // P
    epos = (eslot % P).astype(_f16)
    eqidx = (src % QROWS).astype(np.int16)

    key = (ec * TPC + et) * NQ + eq

    # quarter-major slot layout: q -> t -> blocks. Each quarter's block
    # stream is chopped into full MAXB-block gather instructions
    # independent of tile boundaries.
    slot_start = np.zeros((TPC, NQ), np.int64)
    qbase = np.zeros(NQ + 1, np.int64)
    pos = 0
    for q in range(NQ):
        qbase[q] = pos
        for t in range(TPC):
            slot_start[t, q] = pos
            pos += Btq[t, q] * P
    qbase[NQ] = pos
    S_total = pos
    assert S_total % 128 == 0

    # rank of each edge within its (c,t,q) group
    perm = np.argsort(key, kind="stable")
    gstart = np.concatenate([[0], np.cumsum(np.bincount(key, minlength=NC * TPC * NQ))])[:-1]
    ranks = np.empty(len(perm), np.int64)
    ranks[perm] = np.arange(len(perm)) - gstart[key[perm]]

    flat = slot_start[et, eq] + ranks   # slot within core's flat layout

    qidx_slots = np.zeros((NC, S_total), np.int16)
    dst_slots = np.full((NC, S_total), 999.0, _f16)
    qidx_slots[ec, flat] = eqidx
    dst_slots[ec, flat] = epos

    idx16 = np.empty((NC, P, S_total // 16), np.int16)
    dstl = np.empty((NC, P, S_total // 128), _f16)
    for c in range(NC):
        wrapped = qidx_slots[c].reshape(-1, 16).T   # [16, S/16]
        idx16[c] = np.tile(wrapped, (8, 1))
        dstl[c] = dst_slots[c].reshape(-1, 128).T   # [128, S/128]

    return {
        "Btq": Btq,
        "slot_start": slot_start,
        "S_total": S_total,
        "qbase": qbase,
        "idx16": idx16,
        "dstl": dstl,
        "node_of": node_of,
    }


def _build_program(Btq, slot_start, S_total, qbase, has_bias, repeat=1, loop=1,
                   ablate=""):
    import concourse.bacc as bacc
    import concourse.mybir as mybir
    import concourse.tile as tile
    from contextlib import ExitStack, nullcontext

    f32 = mybir.dt.float32
    f16 = mybir.dt.float16
    nc = bacc.Bacc(
        "TRN2", target_bir_lowering=False, debug=False, num_devices=NC,
        num_swdge_queues=4, dynamic_dma_scratch_size=SCRATCH,
    )

    # xt/out pack two 128-slot tiles per DRAM row pair-wise so every DMA
    # descriptor moves 512B contiguous (fp16 256B rows fall off HBM line
    # rate: writes below 512B do read-modify-write).
    x_d = nc.dram_tensor("x16", [N, D], f16, kind="ExternalInput")
    xt_d = nc.dram_tensor("xt", [SLOTS // 2, 2 * D], f16, kind="ExternalInput")
    idx_d = nc.dram_tensor("idx16", [P, S_total // 16], mybir.dt.int16, kind="ExternalInput")
    dst_d = nc.dram_tensor("dstl", [P, S_total // 128], f16, kind="ExternalInput")
    wt_d = nc.dram_tensor("wt", [D, D], f16, kind="ExternalInput")
    b_d = nc.dram_tensor("bias", [1, D], f32, kind="ExternalInput")
    out_d = nc.dram_tensor("out", [SLOTS // 2, 2 * D], f16, kind="ExternalOutput")

    with tile.TileContext(nc) as tc, ExitStack() as ctx:
        const = ctx.enter_context(tc.tile_pool(name="const", bufs=1))
        gxp = ctx.enter_context(tc.tile_pool(name="gx", bufs=16))
        ohp = ctx.enter_context(tc.tile_pool(name="oh", bufs=12))
        xtp = ctx.enter_context(tc.tile_pool(name="xt", bufs=3))
        htp = ctx.enter_context(tc.tile_pool(name="ht", bufs=3))
        obp = ctx.enter_context(tc.tile_pool(name="ob", bufs=3))
        pag = ctx.enter_context(tc.tile_pool(name="pagg", bufs=4, space="PSUM"))
        pou = ctx.enter_context(tc.tile_pool(name="pout", bufs=2, space="PSUM"))

        idx_t = const.tile([P, S_total // 16], mybir.dt.int16)
        nc.sync.dma_start(out=idx_t[:], in_=idx_d[:])
        dst_t = const.tile([P, S_total // 128], f16)
        nc.sync.dma_start(out=dst_t[:], in_=dst_d[:])
        wt_t = const.tile([D, D], f16)
        nc.sync.dma_start(out=wt_t[:], in_=wt_d[:])
        if has_bias:
            b_t = const.tile([1, D], f32)
            nc.sync.dma_start(out=b_t[:], in_=b_d[:])
            ones_t = const.tile([1, D], f32)
            nc.vector.memset(ones_t[:], 1.0)
        iota_i = const.tile([P, P], mybir.dt.int32)
        nc.gpsimd.iota(iota_i[:], pattern=[[1, P]], base=0, channel_multiplier=0)
        iota_f = const.tile([P, P], f16)
        nc.vector.tensor_copy(out=iota_f[:], in_=iota_i[:])

        maxB = int(Btq.sum(axis=0).max())
        cgx = const.tile([P, maxB, P], f16) if "no_gather" in ablate else None
        coh = const.tile([P, maxB, P], f16) if "no_onehot" in ablate else None
        if cgx is not None:
            nc.vector.memset(cgx[:], 0.25)
        if coh is not None:
            nc.vector.memset(coh[:], 0.0)

        # per-quarter gather instruction schedule: chop each quarter's block
        # stream [qbase[q]/P, qbase[q+1]/P) into MAXB-block instructions.
        qinstr = []  # [q][i] = (c0, nblk)  (chunk-col base, block count)
        for q in range(NQ):
            b0 = int(qbase[q]) // P
            b1 = int(qbase[q + 1]) // P
            qinstr.append(
                [(c, min(MAXB, b1 - c)) for c in range(b0, b1, MAXB)]
            )
        nwave = max(len(qi) for qi in qinstr)
        # chunk col -> (wave, offset within that instruction's gx tile)
        chunk_loc = {}
        for q in range(NQ):
            for w, (c0, nblk) in enumerate(qinstr[q]):
                for j in range(nblk):
                    chunk_loc[c0 + j] = (q, w, j)

        qn = 0
        with (tc.For_i(0, loop) if loop > 1 else nullcontext()):
          for _rep in range(repeat):
            gx_tiles = {}
            oh_tiles = {}

            def issue_wave(w):
                for q in range(NQ):
                    if w >= len(qinstr[q]):
                        continue
                    c0, nblk = qinstr[q][w]
                    if cgx is None:
                        gx = gxp.tile([P, nblk, P], f16, tag="gx")
                        nidx = nblk * P
                        nc.gpsimd.dma_gather(
                            gx[:],
                            x_d[q * QROWS : (q + 1) * QROWS, :],
                            idx_t[:, c0 * 8 : c0 * 8 + nidx // 16],
                            nidx,
                            nidx,
                            D,
                            queue_num=q,
                            single_packet=False,
                        )
                    else:
                        gx = cgx[:, :nblk, :]
                    if "gather_only" not in ablate:
                        if coh is None:
                            oh = ohp.tile([P, nblk, P], f16, tag="oh")
                            nc.vector.tensor_tensor(
                                out=oh[:],
                                in0=iota_f[:].unsqueeze(1).to_broadcast([P, nblk, P]),
                                in1=dst_t[:, c0 : c0 + nblk].unsqueeze(2).to_broadcast([P, nblk, P]),
                                op=mybir.AluOpType.is_equal,
                            )
                        else:
                            oh = coh[:, :nblk, :]
                        oh_tiles[(q, w)] = oh
                    gx_tiles[(q, w)] = gx

            # run 1 wave ahead of consumption (oh pool: 12 bufs = 3 waves)
            issue_wave(0)
            issue_wave(1)
            next_wave = 2
            if "gather_only" in ablate:
                for w in range(2, nwave):
                    issue_wave(w)
                continue
            for t in range(TPC):
                chunks = [(q, b) for q in range(NQ) for b in range(int(Btq[t, q]))]
                need_wave = max(
                    (chunk_loc[int(slot_start[t, q]) // P + b][1] for q, b in chunks),
                    default=-1,
                )
                while next_wave <= min(need_wave + 1, nwave - 1):
                    issue_wave(next_wave)
                    next_wave += 1
                if True:
                    half = t % 2
                    tp = t // 2
                    if half == 0:
                        xt_t = xtp.tile([P, 2 * P], f16, tag="xt")
                        nc.sync.dma_start(
                            out=xt_t[:], in_=xt_d[tp * P : (tp + 1) * P, :]
                        )
                        ob = obp.tile([P, 2 * P], f16, tag="ob")
                        cur_xt, cur_ob = xt_t, ob
                    else:
                        xt_t, ob = cur_xt, cur_ob
                    psum = pag.tile([P, P], f32, space="PSUM", tag="pagg")
                    for i, (q, b) in enumerate(chunks):
                        _, w, boff = chunk_loc[int(slot_start[t, q]) // P + b]
                        nc.tensor.matmul(
                            out=psum[:],
                            lhsT=gx_tiles[(q, w)][:, boff, :],
                            rhs=oh_tiles[(q, w)][:, boff, :],
                            start=(i == 0),
                            stop=(i == len(chunks) - 1),
                        )
                    ht = htp.tile([P, P], f16, tag="ht")
                    if chunks:
                        # h^T = agg^T + x^T (self term)
                        nc.vector.tensor_tensor(
                            out=ht[:],
                            in0=psum[:],
                            in1=xt_t[:, half * P : (half + 1) * P],
                            op=mybir.AluOpType.add,
                        )
                    else:
                        nc.vector.tensor_copy(
                            out=ht[:], in_=xt_t[:, half * P : (half + 1) * P]
                        )
                    po = pou.tile([P, P], f32, space="PSUM", tag="pout")
                    if has_bias:
                        nc.tensor.matmul(out=po[:], lhsT=ht[:], rhs=wt_t[:], start=True, stop=False)
                        nc.tensor.matmul(out=po[:], lhsT=ones_t[:], rhs=b_t[:], start=False, stop=True)
                    else:
                        nc.tensor.matmul(out=po[:], lhsT=ht[:], rhs=wt_t[:], start=True, stop=True)
                    nc.scalar.activation(
                        ob[:, half * P : (half + 1) * P],
                        po[:],
                        mybir.ActivationFunctionType.Relu,
                    )
                    if half == 1:
                        nc.sync.dma_start(
                            out=out_d[tp * P : (tp + 1) * P, :], in_=ob[:]
                        )
    nc.compile()
    return nc


def _prepare(x, edge_index, W, b, repeat=1, loop=1):
    x = np.ascontiguousarray(np.asarray(x, dtype=_f32))
    W = np.asarray(W, dtype=_f32)
    b = np.asarray(b, dtype=_f32)
    pre = _preprocess(edge_index)
    has_bias = bool(np.any(b != 0))
    nc = _build_program(
        pre["Btq"], pre["slot_start"], pre["S_total"], pre["qbase"],
        has_bias, repeat=repeat, loop=loop,
    )
    x16 = x.astype(_f16)
    wt = np.ascontiguousarray(W.T.astype(_f16))
    brow = np.ascontiguousarray(b.reshape(1, D))
    node_of = pre["node_of"]
    in_maps = []
    for c in range(NC):
        nidx = np.where(node_of[c] < 0, 0, node_of[c])
        # x^T per tile pair: [TPC/2, D feat, 2*P nodes] -> [SLOTS/2, 2D]
        xt = np.ascontiguousarray(
            x16[nidx]
            .reshape(TPC // 2, 2, P, D)
            .transpose(0, 3, 1, 2)
            .reshape(SLOTS // 2, 2 * D)
        )
        in_maps.append(
            {
                "x16": x16,
                "xt": xt,
                "idx16": np.ascontiguousarray(pre["idx16"][c]),
                "dstl": np.ascontiguousarray(pre["dstl"][c]),
                "wt": wt,
                "bias": brow,
            }
        )
    return nc, in_maps, node_of


def _assemble(results, node_of):
    out = np.empty((N, D), _f32)
    for c in range(NC):
        oc = (
            results[c]["out"]
            .reshape(TPC // 2, P, 2, D)
            .transpose(0, 2, 1, 3)
            .reshape(SLOTS, D)
        )
        m = node_of[c] >= 0
        out[node_of[c][m]] = oc[m].astype(_f32)
    return out


def kernel(x, edge_index, W, b):
    from concourse.bass_utils import run_bass_kernel_spmd

    nc, in_maps, node_of = _prepare(x, edge_index, W, b)
    res = run_bass_kernel_spmd(nc, in_maps, core_ids=list(range(NC)))
    return _assemble(res.results, node_of)


# revision 38
# speedup vs baseline: 1.1500x; 1.0008x over previous
"""ExpanderGIN message-passing kernel for 8 Trainium2 NeuronCores.

out = relu((x + segment_sum(x[src], dst)) @ W.T + b)

Strategy (graph-parallel, no collectives), fp16 message path:
  - Destination nodes are sharded 8 ways (12500 nodes/core, 98 tiles of
    128 slots). A 4-D bin-packer assigns nodes to tiles so per-(tile,
    src-quarter) in-degree sums land just under 128-block boundaries
    (~3% padding vs ~28% for naive assignment); the block budget B_star
    is shared across cores (SPMD program), per-core loads fit under it.
  - Edge rows are fetched with the SWDGE dma_gather custom instruction
    from a replicated fp16 copy of x. The int16 index limit forces 4
    quarter-tables of 25000 rows. Slots are laid out quarter-major
    (q -> tile -> blocks) and each quarter's block stream is chopped
    into full 1024-index gather instructions independent of tile
    boundaries (80 instructions/rep), one SWDGE queue per quarter,
    issued in waves 1-2 ahead of consumption. single_packet=False.
  - Aggregation: for each 128-edge chunk, a one-hot(dst) matrix [128
    edges, 128 slots] in fp16 is built by comparing an fp16 iota row
    against per-edge dst columns (one broadcast tensor_tensor per gather
    instruction), then TensorE computes agg^T += gx^T @ onehot in PSUM
    (f32). fp16 operands run the PE at 1 cycle/row (vs 4 for f32) with
    fast weight loads.
  - The self term x is added from a host-side permuted/transposed fp16
    copy of x, fused into the PSUM->SBUF eviction add (output fp16 =
    MLP matmul input). xt and out pack two 128-slot tiles per DRAM row
    so every streaming DMA descriptor moves 512B contiguous (256B rows
    fall off HBM line rate).
  - MLP: psum_out[nodes, outfeat] = ht.T @ W^T in fp16 (+ a K=1 bias
    matmul only when b != 0), then ReLU on the scalar engine -> fp16
    out, cast to f32 on the host.
  - Bench note: wall-clock dispatch through axon is ~70ms with ms-level
    jitter, so test.py measures the repeat-slope inside a device-side
    For_i loop (loop=200, repeat 1 vs 3): dispatch cost and loop barrier
    overhead cancel, leaving per-rep kernel time.
"""

import numpy as np

N = 100000
E = 625000
D = 128
NC = 8            # cores
NPC = N // NC     # 12500 nodes per core
P = 128
TPC = (NPC + P - 1) // P   # 98 tiles per core
SLOTS = TPC * P            # 12544 slots per core
NQ = 4                     # quarter tables (int16 index limit)
QROWS = N // NQ            # 25000

SCRATCH = 49152            # SWDGE ring: 3072 descs/queue = 1.5 of the
                           # 2048-idx gathers (partial gen/drain overlap)
MAXI = 2048                # >1024 idx needs single_packet=False (64-desc
                           # packet ceiling per engine); 2048 = best measured
MAXB = MAXI // P           # blocks per gather instruction

_f32 = np.float32
_f16 = np.float16


def _pack_tiles(dq):
    """4-D bin-packing: assign each core's nodes to 98 tiles of <=128 nodes
    so per-(tile, quarter) in-degree sums fit a shared block budget B_star
    [TPC, NQ] with minimal total padding. Returns (B_star, node_of)."""
    Tcq = dq.reshape(NC, NPC, NQ).sum(axis=1)          # [NC, NQ] totals
    need = (Tcq.max(axis=0) + P - 1) // P              # blocks per quarter

    for margin in (2, 3, 4, 6, 9):
        Kq = need + margin
        # distribute Kq[q] blocks over TPC tiles (>=1 each), extras spread
        # round-robin so per-tile totals stay balanced
        B_star = np.ones((TPC, NQ), np.int64)
        extras = []
        for q in range(NQ):
            extras += [q] * int(Kq[q] - TPC)
        for i, q in enumerate(extras):
            B_star[i % TPC, q] += 1
        caps0 = B_star * P

        node_of = np.full((NC, SLOTS), -1, np.int64)
        ok = True
        for c in range(NC):
            nodes = np.arange(c * NPC, (c + 1) * NPC)
            d = dq[nodes]                              # [NPC, NQ]
            order = np.argsort(-d.sum(1), kind="stable")
            rem = caps0.astype(np.int64).copy()
            cnt = np.zeros(TPC, np.int64)
            assign = np.empty(NPC, np.int64)
            for n in order:
                dn = d[n]
                feas = (cnt < P) & np.all(rem >= dn, axis=1)
                if not feas.any():
                    ok = False
                    break
                score = (rem - dn).min(axis=1) * 1024 + (P - cnt)
                score[~feas] = -1
                t = int(np.argmax(score))
                assign[n] = t
                rem[t] -= dn
                cnt[t] += 1
            if not ok:
                break
            fill = np.zeros(TPC, np.int64)
            for n in range(NPC):
                t = assign[n]
                node_of[c, t * P + fill[t]] = nodes[n]
                fill[t] += 1
        if ok:
            return B_star, node_of
    raise RuntimeError("tile packing failed at all margins")


def _preprocess(edge_index):
    """Shard edges. Returns per-core host arrays + layout metadata."""
    src = np.asarray(edge_index[0]).astype(np.int64)
    dst = np.asarray(edge_index[1]).astype(np.int64)

    eq = src // QROWS
    # per-node quarter in-degree
    dq = np.bincount(dst * NQ + eq, minlength=N * NQ).reshape(N, NQ)

    Btq, node_of = _pack_tiles(dq)
    slot_of = np.empty(N, np.int64)
    for c in range(NC):
        m = node_of[c] >= 0
        slot_of[node_of[c][m]] = np.nonzero(m)[0]

    ec = dst // NPC
    eslot = slot_of[dst]
    et = eslot // P
    epos = (eslot % P).astype(_f16)
    eqidx = (src % QROWS).astype(np.int16)

    key = (ec * TPC + et) * NQ + eq

    # quarter-major slot layout: q -> t -> blocks. Each quarter's block
    # stream is chopped into full MAXB-block gather instructions
    # independent of tile boundaries.
    slot_start = np.zeros((TPC, NQ), np.int64)
    qbase = np.zeros(NQ + 1, np.int64)
    pos = 0
    for q in range(NQ):
        qbase[q] = pos
        for t in range(TPC):
            slot_start[t, q] = pos
            pos += Btq[t, q] * P
    qbase[NQ] = pos
    S_total = pos
    assert S_total % 128 == 0

    # rank of each edge within its (c,t,q) group
    perm = np.argsort(key, kind="stable")
    gstart = np.concatenate([[0], np.cumsum(np.bincount(key, minlength=NC * TPC * NQ))])[:-1]
    ranks = np.empty(len(perm), np.int64)
    ranks[perm] = np.arange(len(perm)) - gstart[key[perm]]

    flat = slot_start[et, eq] + ranks   # slot within core's flat layout

    qidx_slots = np.zeros((NC, S_total), np.int16)
    dst_slots = np.full((NC, S_total), 999.0, _f16)
    qidx_slots[ec, flat] = eqidx
    dst_slots[ec, flat] = epos

    idx16 = np.empty((NC, P, S_total // 16), np.int16)
    dstl = np.empty((NC, P, S_total // 128), _f16)
    for c in range(NC):
        wrapped = qidx_slots[c].reshape(-1, 16).T   # [16, S/16]
        idx16[c] = np.tile(wrapped, (8, 1))
        dstl[c] = dst_slots[c].reshape(-1, 128).T   # [128, S/128]

    return {
        "Btq": Btq,
        "slot_start": slot_start,
        "S_total": S_total,
        "qbase": qbase,
        "idx16": idx16,
        "dstl": dstl,
        "node_of": node_of,
    }


def _build_program(Btq, slot_start, S_total, qbase, has_bias, repeat=1, loop=1,
                   ablate=""):
    import concourse.bacc as bacc
    import concourse.mybir as mybir
    import concourse.tile as tile
    from contextlib import ExitStack, nullcontext

    f32 = mybir.dt.float32
    f16 = mybir.dt.float16
    nc = bacc.Bacc(
        "TRN2", target_bir_lowering=False, debug=False, num_devices=NC,
        num_swdge_queues=4, dynamic_dma_scratch_size=SCRATCH,
    )

    # xt/out pack two 128-slot tiles per DRAM row pair-wise so every DMA
    # descriptor moves 512B contiguous (fp16 256B rows fall off HBM line
    # rate: writes below 512B do read-modify-write).
    x_d = nc.dram_tensor("x16", [N, D], f16, kind="ExternalInput")
    xt_d = nc.dram_tensor("xt", [SLOTS // 2, 2 * D], f16, kind="ExternalInput")
    idx_d = nc.dram_tensor("idx16", [P, S_total // 16], mybir.dt.int16, kind="ExternalInput")
    dst_d = nc.dram_tensor("dstl", [P, S_total // 128], f16, kind="ExternalInput")
    wt_d = nc.dram_tensor("wt", [D, D], f16, kind="ExternalInput")
    b_d = nc.dram_tensor("bias", [1, D], f32, kind="ExternalInput")
    out_d = nc.dram_tensor("out", [SLOTS // 2, 2 * D], f16, kind="ExternalOutput")

    with tile.TileContext(nc) as tc, ExitStack() as ctx:
        const = ctx.enter_context(tc.tile_pool(name="const", bufs=1))
        gxp = ctx.enter_context(tc.tile_pool(name="gx", bufs=16))
        ohp = ctx.enter_context(tc.tile_pool(name="oh", bufs=12))
        xtp = ctx.enter_context(tc.tile_pool(name="xt", bufs=3))
        htp = ctx.enter_context(tc.tile_pool(name="ht", bufs=3))
        obp = ctx.enter_context(tc.tile_pool(name="ob", bufs=3))
        pag = ctx.enter_context(tc.tile_pool(name="pagg", bufs=4, space="PSUM"))
        pou = ctx.enter_context(tc.tile_pool(name="pout", bufs=2, space="PSUM"))

        idx_t = const.tile([P, S_total // 16], mybir.dt.int16)
        nc.sync.dma_start(out=idx_t[:], in_=idx_d[:])
        dst_t = const.tile([P, S_total // 128], f16)
        nc.sync.dma_start(out=dst_t[:], in_=dst_d[:])
        wt_t = const.tile([D, D], f16)
        nc.sync.dma_start(out=wt_t[:], in_=wt_d[:])
        if has_bias:
            b_t = const.tile([1, D], f32)
            nc.sync.dma_start(out=b_t[:], in_=b_d[:])
            ones_t = const.tile([1, D], f32)
            nc.vector.memset(ones_t[:], 1.0)
        iota_i = const.tile([P, P], mybir.dt.int32)
        nc.gpsimd.iota(iota_i[:], pattern=[[1, P]], base=0, channel_multiplier=0)
        iota_f = const.tile([P, P], f16)
        nc.vector.tensor_copy(out=iota_f[:], in_=iota_i[:])

        maxB = int(Btq.sum(axis=0).max())
        cgx = const.tile([P, maxB, P], f16) if "no_gather" in ablate else None
        coh = const.tile([P, maxB, P], f16) if "no_onehot" in ablate else None
        if cgx is not None:
            nc.vector.memset(cgx[:], 0.25)
        if coh is not None:
            nc.vector.memset(coh[:], 0.0)

        # per-quarter gather instruction schedule: chop each quarter's block
        # stream [qbase[q]/P, qbase[q+1]/P) into MAXB-block instructions.
        qinstr = []  # [q][i] = (c0, nblk)  (chunk-col base, block count)
        for q in range(NQ):
            b0 = int(qbase[q]) // P
            b1 = int(qbase[q + 1]) // P
            qinstr.append(
                [(c, min(MAXB, b1 - c)) for c in range(b0, b1, MAXB)]
            )
        nwave = max(len(qi) for qi in qinstr)
        # chunk col -> (wave, offset within that instruction's gx tile)
        chunk_loc = {}
        for q in range(NQ):
            for w, (c0, nblk) in enumerate(qinstr[q]):
                for j in range(nblk):
                    chunk_loc[c0 + j] = (q, w, j)

        qn = 0
        with (tc.For_i(0, loop) if loop > 1 else nullcontext()):
          for _rep in range(repeat):
            gx_tiles = {}
            oh_tiles = {}

            def issue_wave(w):
                for q in range(NQ):
                    if w >= len(qinstr[q]):
                        continue
                    c0, nblk = qinstr[q][w]
                    if cgx is None:
                        gx = gxp.tile([P, nblk, P], f16, tag="gx")
                        nidx = nblk * P
                        nc.gpsimd.dma_gather(
                            gx[:],
                            x_d[q * QROWS : (q + 1) * QROWS, :],
                            idx_t[:, c0 * 8 : c0 * 8 + nidx // 16],
                            nidx,
                            nidx,
                            D,
                            queue_num=q,
                            single_packet=False,
                        )
                    else:
                        gx = cgx[:, :nblk, :]
                    if "gather_only" not in ablate:
                        if coh is None:
                            oh = ohp.tile([P, nblk, P], f16, tag="oh")
                            nc.vector.tensor_tensor(
                                out=oh[:],
                                in0=iota_f[:].unsqueeze(1).to_broadcast([P, nblk, P]),
                                in1=dst_t[:, c0 : c0 + nblk].unsqueeze(2).to_broadcast([P, nblk, P]),
                                op=mybir.AluOpType.is_equal,
                            )
                        else:
                            oh = coh[:, :nblk, :]
                        oh_tiles[(q, w)] = oh
                    gx_tiles[(q, w)] = gx

            # run 1 wave ahead of consumption (oh pool: 12 bufs = 3 waves)
            issue_wave(0)
            issue_wave(1)
            next_wave = 2
            if "gather_only" in ablate:
                for w in range(2, nwave):
                    issue_wave(w)
                continue
            for t in range(TPC):
                chunks = [(q, b) for q in range(NQ) for b in range(int(Btq[t, q]))]
                need_wave = max(
                    (chunk_loc[int(slot_start[t, q]) // P + b][1] for q, b in chunks),
                    default=-1,
                )
                while next_wave <= min(need_wave + 1, nwave - 1):
                    issue_wave(next_wave)
                    next_wave += 1
                if True:
                    half = t % 2
                    tp = t // 2
                    if half == 0:
                        xt_t = xtp.tile([P, 2 * P], f16, tag="xt")
                        nc.sync.dma_start(
                            out=xt_t[:], in_=xt_d[tp * P : (tp + 1) * P, :]
                        )
                        ob = obp.tile([P, 2 * P], f16, tag="ob")
                        cur_xt, cur_ob = xt_t, ob
                    else:
                        xt_t, ob = cur_xt, cur_ob
                    psum = pag.tile([P, P], f32, space="PSUM", tag="pagg")
                    for i, (q, b) in enumerate(chunks):
                        _, w, boff = chunk_loc[int(slot_start[t, q]) // P + b]
                        nc.tensor.matmul(
                            out=psum[:],
                            lhsT=gx_tiles[(q, w)][:, boff, :],
                            rhs=oh_tiles[(q, w)][:, boff, :],
                            start=(i == 0),
                            stop=(i == len(chunks) - 1),
                        )
                    ht = htp.tile([P, P], f16, tag="ht")
                    if chunks:
                        # h^T = agg^T + x^T (self term)
                        nc.vector.tensor_tensor(
                            out=ht[:],
                            in0=psum[:],
                            in1=xt_t[:, half * P : (half + 1) * P],
                            op=mybir.AluOpType.add,
                        )
                    else:
                        nc.vector.tensor_copy(
                            out=ht[:], in_=xt_t[:, half * P : (half + 1) * P]
                        )
                    po = pou.tile([P, P], f32, space="PSUM", tag="pout")
                    if has_bias:
                        nc.tensor.matmul(out=po[:], lhsT=ht[:], rhs=wt_t[:], start=True, stop=False)
                        nc.tensor.matmul(out=po[:], lhsT=ones_t[:], rhs=b_t[:], start=False, stop=True)
                    else:
                        nc.tensor.matmul(out=po[:], lhsT=ht[:], rhs=wt_t[:], start=True, stop=True)
                    nc.scalar.activation(
                        ob[:, half * P : (half + 1) * P],
                        po[:],
                        mybir.ActivationFunctionType.Relu,
                    )
                    if half == 1:
                        nc.sync.dma_start(
                            out=out_d[tp * P : (tp + 1) * P, :], in_=ob[:]
                        )
    nc.compile()
    return nc


def _prepare(x, edge_index, W, b, repeat=1, loop=1):
    x = np.ascontiguousarray(np.asarray(x, dtype=_f32))
    W = np.asarray(W, dtype=_f32)
    b = np.asarray(b, dtype=_f32)
    pre = _preprocess(edge_index)
    has_bias = bool(np.any(b != 0))
    nc = _build_program(
        pre["Btq"], pre["slot_start"], pre["S_total"], pre["qbase"],
        has_bias, repeat=repeat, loop=loop,
    )
    x16 = x.astype(_f16)
    wt = np.ascontiguousarray(W.T.astype(_f16))
    brow = np.ascontiguousarray(b.reshape(1, D))
    node_of = pre["node_of"]
    in_maps = []
    for c in range(NC):
        nidx = np.where(node_of[c] < 0, 0, node_of[c])
        # x^T per tile pair: [TPC/2, D feat, 2*P nodes] -> [SLOTS/2, 2D]
        xt = np.ascontiguousarray(
            x16[nidx]
            .reshape(TPC // 2, 2, P, D)
            .transpose(0, 3, 1, 2)
            .reshape(SLOTS // 2, 2 * D)
        )
        in_maps.append(
            {
                "x16": x16,
                "xt": xt,
                "idx16": np.ascontiguousarray(pre["idx16"][c]),
                "dstl": np.ascontiguousarray(pre["dstl"][c]),
                "wt": wt,
                "bias": brow,
            }
        )
    return nc, in_maps, node_of


def _assemble(results, node_of):
    out = np.empty((N, D), _f32)
    for c in range(NC):
        oc = (
            results[c]["out"]
            .reshape(TPC // 2, P, 2, D)
            .transpose(0, 2, 1, 3)
            .reshape(SLOTS, D)
        )
        m = node_of[c] >= 0
        out[node_of[c][m]] = oc[m].astype(_f32)
    return out


def kernel(x, edge_index, W, b):
    from concourse.bass_utils import run_bass_kernel_spmd

    nc, in_maps, node_of = _prepare(x, edge_index, W, b)
    res = run_bass_kernel_spmd(nc, in_maps, core_ids=list(range(NC)))
    return _assemble(res.results, node_of)
